# revision 16
# baseline (speedup 1.0000x reference)
"""Bass/Trainium2 kernel for nn_ConexaoEsparsa (block-sparse einsum).

Computes out[b,o,d,s] = sum_i x[b,i] * peso[o,i,d,s] * mascara[o,i]
 = (256,1024) @ (1024, 512*16*8) matmul whose weight is 90% block-sparse:
for each (o,i), the 16*8=128-element (d,s) block is kept iff mascara[o,i]!=0.

Strategy (8 NeuronCores, tensor-parallel over o = out_features):
  - Shard o into 8 slices of 64; replicate x.
  - Host precomputes, per o, the list of nonzero-i indices from mascara
    (sparsity pattern only; all values flow through the device).
  - Device: dma_gather fetches the ~102 nonzero 512B peso blocks per o
    (padded to K=128) and the matching rows of x^T (pad rows point at an
    appended zero row, so padding contributes exactly 0).
  - One fp32r matmul per o: psum[ds=128, b=256] = pesoC[k,ds].T @ xg[k,b]
    (free dim 256 => full-rate fp32r), then DVE copy psum->sbuf and a
    batched DMA to the (ds, o, b)-layout output in DRAM.
  - Host reassembles/transposes the 8 core outputs into (b, o, d, s).
"""

import numpy as np

_B, _IN, _OUT, _D, _S = 256, 1024, 512, 16, 8
_NCORES = 8
_OL = _OUT // _NCORES          # 64 output features per core
_DS = _D * _S                  # 128
_G = 8                         # o's per gather/store group
_NG = _OL // _G                # 8 groups per core

_prog_cache = {}
_last_results = None


def _build_program(n_mm):
    import concourse.mybir as mybir
    import concourse.tile as tile
    from concourse import bacc

    nc = bacc.Bacc("TRN2", target_bir_lowering=False, debug=False,
                   num_devices=_NCORES)
    f32 = mybir.dt.float32
    f32r = mybir.dt.float32r
    i16 = mybir.dt.int16

    nv = _G * n_mm                 # index blocks of 128 per group
    nidx = nv * 128                # gather indices per group
    icols = nidx // 16             # idx tile columns (16-partition wrap)

    # Tables are f32r: the f32r-typed gather rounds to the fp32r format the
    # PE requires of its operands (the BIR verifier rejects non-rounded
    # producers feeding an fp32r matmul).
    peso = nc.dram_tensor("peso", [_OL * _IN, _DS], f32r, kind="ExternalInput")
    xTz = nc.dram_tensor("xTz", [_IN + 1, _B], f32r, kind="ExternalInput")
    pidx = nc.dram_tensor("pidx", [_NG, 128, icols], i16, kind="ExternalInput")
    xidx = nc.dram_tensor("xidx", [_NG, 128, icols], i16, kind="ExternalInput")
    outT = nc.dram_tensor("outT", [_DS, _OL, _B], f32, kind="ExternalOutput")

    with tile.TileContext(nc) as tc:
        with (
            tc.tile_pool(name="idx", bufs=3) as idxp,
            tc.tile_pool(name="pw", bufs=3) as pwp,
            tc.tile_pool(name="xw", bufs=3) as xwp,
            tc.tile_pool(name="ob", bufs=3) as obp,
            tc.tile_pool(name="ps", bufs=8, space="PSUM") as psp,
        ):
            for grp in range(_NG):
                pidx_t = idxp.tile([128, icols], i16, tag="pidx")
                xidx_t = idxp.tile([128, icols], i16, tag="xidx")
                nc.sync.dma_start(out=pidx_t[:], in_=pidx[grp])
                nc.sync.dma_start(out=xidx_t[:], in_=xidx[grp])

                pesoC = pwp.tile([128, nv * _DS], f32r)
                xg = xwp.tile([128, nv * _B], f32r)
                nc.gpsimd.dma_gather(
                    pesoC[:].rearrange("p (v e) -> p v e", v=nv),
                    peso[grp * _G * _IN:(grp + 1) * _G * _IN, :],
                    pidx_t[:],
                    nidx, nidx, _DS,
                )
                nc.gpsimd.dma_gather(
                    xg[:].rearrange("p (v e) -> p v e", v=nv),
                    xTz[:],
                    xidx_t[:],
                    nidx, nidx, _B,
                )

                outsb = obp.tile([_DS, _G * _B], f32)
                for g in range(_G):
                    psum = psp.tile([_DS, _B], f32)
                    for m in range(n_mm):
                        c = g * n_mm + m
                        nc.tensor.matmul(
                            psum[:],
                            lhsT=pesoC[:, c * _DS:(c + 1) * _DS],
                            rhs=xg[:, c * _B:(c + 1) * _B],
                            start=(m == 0), stop=(m == n_mm - 1),
                        )
                    nc.vector.tensor_copy(
                        out=outsb[:, g * _B:(g + 1) * _B], in_=psum[:])
                nc.sync.dma_start(
                    out=outT[:, grp * _G:(grp + 1) * _G, :], in_=outsb[:])

    nc.compile()
    return nc


def _get_program(n_mm):
    if n_mm not in _prog_cache:
        _prog_cache[n_mm] = _build_program(n_mm)
    return _prog_cache[n_mm]


def _wrap16(flat):
    """[n] -> [128, n//16] int16 tile image: index j at [j % 16, j // 16],
    replicated into all 8 16-partition groups (one copy per Q7 core)."""
    n = flat.shape[0]
    arr = np.empty((128, n // 16), np.int16)
    blk = flat.reshape(n // 16, 16).T
    for k in range(8):
        arr[16 * k:16 * (k + 1)] = blk
    return arr


def _prep_in_maps(x, peso, mascara):
    x = np.asarray(x, dtype=np.float32)
    peso = np.asarray(peso, dtype=np.float32)
    mascara = np.asarray(mascara, dtype=np.float32)
    assert x.shape == (_B, _IN) and peso.shape == (_OUT, _IN, _D, _S)

    # x^T with an extra all-zero row used as the gather target for padding.
    xTz = np.zeros((_IN + 1, _B), np.float32)
    xTz[:_IN] = x.T

    max_nnz = int((mascara != 0).sum(axis=1).max())
    n_mm = max(1, -(-max_nnz // 128))  # 128-row matmul chunks per o
    kpad = 128 * n_mm
    nv = _G * n_mm
    nidx = nv * 128
    icols = nidx // 16

    in_maps = []
    for c in range(_NCORES):
        o0 = c * _OL
        peso_c = peso[o0:o0 + _OL].reshape(_OL * _IN, _DS)
        pidx = np.zeros((_NG, 128, icols), np.int16)
        xidx = np.zeros((_NG, 128, icols), np.int16)
        for grp in range(_NG):
            pflat = np.zeros(nidx, np.int32)
            xflat = np.full(nidx, _IN, np.int32)
            for g in range(_G):
                ol = grp * _G + g
                nz = np.nonzero(mascara[o0 + ol])[0].astype(np.int32)
                ppad = np.zeros(kpad, np.int32) + (ol % _G) * _IN
                ppad[:len(nz)] = (ol % _G) * _IN + nz
                xpad = np.full(kpad, _IN, np.int32)
                xpad[:len(nz)] = nz
                s = g * n_mm * 128
                pflat[s:s + kpad] = ppad
                xflat[s:s + kpad] = xpad
            pidx[grp] = _wrap16(pflat.astype(np.int16))
            xidx[grp] = _wrap16(xflat.astype(np.int16))
        in_maps.append({
            "peso": peso_c,
            "xTz": xTz,
            "pidx": pidx,
            "xidx": xidx,
        })
    return in_maps, n_mm


def kernel(x, peso, mascara):
    from concourse.bass_utils import run_bass_kernel_spmd

    in_maps, n_mm = _prep_in_maps(x, peso, mascara)
    nc = _get_program(n_mm)
    res = run_bass_kernel_spmd(nc, in_maps, list(range(_NCORES)))
    global _last_results
    _last_results = res

    # outT per core: [DS, OL, B]; concat over o then reorder to (b, o, d, s).
    full = np.concatenate(
        [res.results[c]["outT"] for c in range(_NCORES)], axis=1)
    out = full.transpose(2, 1, 0).reshape(_B, _OUT, _D, _S)
    return np.ascontiguousarray(out)


# revision 18
# speedup vs baseline: 1.0449x; 1.0449x over previous
"""Bass/Trainium2 kernel for nn_ConexaoEsparsa (block-sparse einsum).

Computes out[b,o,d,s] = sum_i x[b,i] * peso[o,i,d,s] * mascara[o,i]
 = (256,1024) @ (1024, 512*16*8) matmul whose weight is 90% block-sparse:
for each (o,i), the 16*8=128-element (d,s) block is kept iff mascara[o,i]!=0.

Strategy (8 NeuronCores, tensor-parallel over o = out_features):
  - Shard o into 8 slices of 64; replicate x.
  - Host precomputes, per o, the list of nonzero-i indices from mascara
    (sparsity pattern only; all values flow through the device).
  - Device: dma_gather fetches the ~102 nonzero 512B peso blocks per o
    (padded to K=128) and the matching rows of x^T (pad rows point at an
    appended zero row, so padding contributes exactly 0).
  - One fp32r matmul per o: psum[ds=128, b=256] = pesoC[k,ds].T @ xg[k,b]
    (free dim 256 => full-rate fp32r), then DVE copy psum->sbuf and a
    batched DMA to the (ds, o, b)-layout output in DRAM.
  - Host reassembles/transposes the 8 core outputs into (b, o, d, s).
"""

import numpy as np

_B, _IN, _OUT, _D, _S = 256, 1024, 512, 16, 8
_NCORES = 8
_OL = _OUT // _NCORES          # 64 output features per core
_DS = _D * _S                  # 128
_G = 8                         # o's per gather/store group
_NG = _OL // _G                # 8 groups per core

_prog_cache = {}
_last_results = None


def _build_program(n_mm):
    import concourse.mybir as mybir
    import concourse.tile as tile
    from concourse import bacc

    nc = bacc.Bacc("TRN2", target_bir_lowering=False, debug=False,
                   num_devices=_NCORES)
    f32 = mybir.dt.float32
    f32r = mybir.dt.float32r
    i16 = mybir.dt.int16

    nv = _G * n_mm                 # index blocks of 128 per group
    nidx = nv * 128                # gather indices per group
    icols = nidx // 16             # idx tile columns (16-partition wrap)

    # Tables are f32r: the f32r-typed gather rounds to the fp32r format the
    # PE requires of its operands (the BIR verifier rejects non-rounded
    # producers feeding an fp32r matmul).
    peso = nc.dram_tensor("peso", [_OL * _IN, _DS], f32r, kind="ExternalInput")
    xTz = nc.dram_tensor("xTz", [_IN + 1, _B], f32r, kind="ExternalInput")
    pidx = nc.dram_tensor("pidx", [_NG, 128, icols], i16, kind="ExternalInput")
    xidx = nc.dram_tensor("xidx", [_NG, 128, icols], i16, kind="ExternalInput")
    outT = nc.dram_tensor("outT", [_DS, _OL, _B], f32, kind="ExternalOutput")

    with tile.TileContext(nc) as tc:
        # bufs=NG on the gather pools lets every group's gathers queue on the
        # Q7 back-to-back (the serial descriptor-emission chain is the
        # bottleneck); MMs/copies/stores trail behind the chain.
        with (
            tc.tile_pool(name="idx", bufs=_NG) as idxp,
            tc.tile_pool(name="pw", bufs=_NG) as pwp,
            tc.tile_pool(name="xw", bufs=_NG) as xwp,
            tc.tile_pool(name="ob", bufs=3) as obp,
            tc.tile_pool(name="ps", bufs=8, space="PSUM") as psp,
        ):
            for grp in range(_NG):
                pidx_t = idxp.tile([128, icols], i16, tag="pidx")
                xidx_t = idxp.tile([128, icols], i16, tag="xidx")
                nc.sync.dma_start(out=pidx_t[:], in_=pidx[grp])
                nc.sync.dma_start(out=xidx_t[:], in_=xidx[grp])

                pesoC = pwp.tile([128, nv * _DS], f32r)
                xg = xwp.tile([128, nv * _B], f32r)
                nc.gpsimd.dma_gather(
                    pesoC[:].rearrange("p (v e) -> p v e", v=nv),
                    peso[grp * _G * _IN:(grp + 1) * _G * _IN, :],
                    pidx_t[:],
                    nidx, nidx, _DS,
                )
                nc.gpsimd.dma_gather(
                    xg[:].rearrange("p (v e) -> p v e", v=nv),
                    xTz[:],
                    xidx_t[:],
                    nidx, nidx, _B,
                )

                outsb = obp.tile([_DS, _G * _B], f32)
                for g in range(_G):
                    psum = psp.tile([_DS, _B], f32)
                    for m in range(n_mm):
                        c = g * n_mm + m
                        nc.tensor.matmul(
                            psum[:],
                            lhsT=pesoC[:, c * _DS:(c + 1) * _DS],
                            rhs=xg[:, c * _B:(c + 1) * _B],
                            start=(m == 0), stop=(m == n_mm - 1),
                        )
                    if g % 2 == 0:
                        nc.vector.tensor_copy(
                            out=outsb[:, g * _B:(g + 1) * _B], in_=psum[:])
                    else:
                        nc.scalar.copy(
                            out=outsb[:, g * _B:(g + 1) * _B], in_=psum[:])
                nc.sync.dma_start(
                    out=outT[:, grp * _G:(grp + 1) * _G, :], in_=outsb[:])

    nc.compile()
    return nc


def _get_program(n_mm):
    if n_mm not in _prog_cache:
        _prog_cache[n_mm] = _build_program(n_mm)
    return _prog_cache[n_mm]


def _wrap16(flat):
    """[n] -> [128, n//16] int16 tile image: index j at [j % 16, j // 16],
    replicated into all 8 16-partition groups (one copy per Q7 core)."""
    n = flat.shape[0]
    arr = np.empty((128, n // 16), np.int16)
    blk = flat.reshape(n // 16, 16).T
    for k in range(8):
        arr[16 * k:16 * (k + 1)] = blk
    return arr


def _prep_in_maps(x, peso, mascara):
    x = np.asarray(x, dtype=np.float32)
    peso = np.asarray(peso, dtype=np.float32)
    mascara = np.asarray(mascara, dtype=np.float32)
    assert x.shape == (_B, _IN) and peso.shape == (_OUT, _IN, _D, _S)

    # x^T with an extra all-zero row used as the gather target for padding.
    xTz = np.zeros((_IN + 1, _B), np.float32)
    xTz[:_IN] = x.T

    max_nnz = int((mascara != 0).sum(axis=1).max())
    n_mm = max(1, -(-max_nnz // 128))  # 128-row matmul chunks per o
    kpad = 128 * n_mm
    nv = _G * n_mm
    nidx = nv * 128
    icols = nidx // 16

    in_maps = []
    for c in range(_NCORES):
        o0 = c * _OL
        peso_c = peso[o0:o0 + _OL].reshape(_OL * _IN, _DS)
        pidx = np.zeros((_NG, 128, icols), np.int16)
        xidx = np.zeros((_NG, 128, icols), np.int16)
        for grp in range(_NG):
            pflat = np.zeros(nidx, np.int32)
            xflat = np.full(nidx, _IN, np.int32)
            for g in range(_G):
                ol = grp * _G + g
                nz = np.nonzero(mascara[o0 + ol])[0].astype(np.int32)
                ppad = np.zeros(kpad, np.int32) + (ol % _G) * _IN
                ppad[:len(nz)] = (ol % _G) * _IN + nz
                xpad = np.full(kpad, _IN, np.int32)
                xpad[:len(nz)] = nz
                s = g * n_mm * 128
                pflat[s:s + kpad] = ppad
                xflat[s:s + kpad] = xpad
            pidx[grp] = _wrap16(pflat.astype(np.int16))
            xidx[grp] = _wrap16(xflat.astype(np.int16))
        in_maps.append({
            "peso": peso_c,
            "xTz": xTz,
            "pidx": pidx,
            "xidx": xidx,
        })
    return in_maps, n_mm


def kernel(x, peso, mascara):
    from concourse.bass_utils import run_bass_kernel_spmd

    in_maps, n_mm = _prep_in_maps(x, peso, mascara)
    nc = _get_program(n_mm)
    res = run_bass_kernel_spmd(nc, in_maps, list(range(_NCORES)))
    global _last_results
    _last_results = res

    # outT per core: [DS, OL, B]; concat over o then reorder to (b, o, d, s).
    full = np.concatenate(
        [res.results[c]["outT"] for c in range(_NCORES)], axis=1)
    out = full.transpose(2, 1, 0).reshape(_B, _OUT, _D, _S)
    return np.ascontiguousarray(out)
